# revision 1
# baseline (speedup 1.0000x reference)
"""DCNv4 Trainium2 Bass kernel (8-core data parallel).

Sharding: 8 cores = 4 images x 2 H-halves (64 rows each + 2-row halo).
Per core, layouts keep channels-or-w in partitions:
  feat [c, (h,w)] bf16  <- conv 1x1 GEMM (f32r, stationary conv_w.T)
  V    [w, (h,c)] bf16  <- value GEMM per row
  om   [w, 108] PSUM    <- offset/mask GEMM per row (ox36|oy36|m36)
DCN core = 25-tap dynamic conv. With |offset| < 1 the bilinear weights
are tents: w[s] = relu(1-|o-s|); 9 points x 3x3 tents bin into a 5x5
stencil (bins, 100 vals/px) via zero-slotted products + one reduce.
The x-shift of the stencil is done with PE matmuls, not DMA:
  bins [w,100] --(mm ident)--> binsT [100, w] --evac--> padded SBUF
  [100, 132] (2 zero cols each side), then 5 matmuls with a constant
  duplication matrix R_dx [100, 40] produce binsS[w, (dx,dy,g,dup2)] =
  bins[w-dx+2, ...], each value twice (dup2 makes the product's inner
  dim packed so DVE runs at 2x in bf16). Zero pad cols = x-edge mask.
Products prod[w, (dy,dx,c)] = V[w, h+dy, c] * binsS (5 DVE ops, bf16
2x), then 25 accumulating PE matmuls (ident rhs) transpose+sum the
taps into dp[c, w]; all matmul streams are bf16 (1 cycle/row).
"""

import sys
from contextlib import ExitStack

for _p in ("/opt/trn_rl_repo",):
    if _p not in sys.path:
        sys.path.insert(0, _p)

import numpy as np

import concourse.bass as bass
import concourse.bacc as bacc
import concourse.tile as tile
from concourse import mybir
from concourse.bass_utils import run_bass_kernel_spmd

F32 = mybir.dt.float32
F32R = mybir.dt.float32r
BF16 = mybir.dt.bfloat16
ALU = mybir.AluOpType
AF = mybir.ActivationFunctionType
AX = mybir.AxisListType

N, C, H, W = 4, 128, 128, 128
G, K = 4, 9
OM_DIM = 112
OMP = 108  # permuted om rows actually used: ox36 | oy36 | m36
HS = 64    # own rows per core
HH = HS + 4  # with 2-row halo each side
NCORES = 8

_CACHE = {}


def _ap(t, offset, pattern):
    return bass.AP(tensor=t, offset=offset, ap=[list(p) for p in pattern])


def _build_program(debug=False):
    nc = bacc.Bacc("TRN2", target_bir_lowering=False, debug=False,
                   num_devices=NCORES)
    xs = nc.dram_tensor("xs", [C, HH, W], F32, kind="ExternalInput").ap()
    cwT = nc.dram_tensor("cwT", [C, C], F32, kind="ExternalInput").ap()
    vwT = nc.dram_tensor("vwT", [C, C], BF16, kind="ExternalInput").ap()
    owT = nc.dram_tensor("owT", [C, OMP], BF16, kind="ExternalInput").ap()
    outwT = nc.dram_tensor("outwT", [C, C], BF16, kind="ExternalInput").ap()
    bconv = nc.dram_tensor("bconv", [C, 1], F32, kind="ExternalInput").ap()
    bout = nc.dram_tensor("bout", [C, 1], F32, kind="ExternalInput").ap()
    ident = nc.dram_tensor("ident", [C, C], BF16, kind="ExternalInput").ap()
    rdup = nc.dram_tensor("rdup", [100, 100], BF16, kind="ExternalInput").ap()
    sid = nc.dram_tensor("sid", [C, 5 * C], BF16, kind="ExternalInput").ap()
    y = nc.dram_tensor("y", [C, HS, W], F32, kind="ExternalOutput").ap()
    dbg = {}
    if debug:
        dbg["feat"] = nc.dram_tensor("dbg_feat", [C, HH, W], BF16,
                                     kind="ExternalOutput").ap()
        dbg["v"] = nc.dram_tensor("dbg_v", [W, HH, C], BF16,
                                  kind="ExternalOutput").ap()
        dbg["binss"] = nc.dram_tensor("dbg_binss", [W, HS, 100], BF16,
                                      kind="ExternalOutput").ap()
        dbg["dcn"] = nc.dram_tensor("dbg_dcn", [C, HS, W], BF16,
                                    kind="ExternalOutput").ap()

    with tile.TileContext(nc) as tc:
        with ExitStack() as ctx:
            _kernel_body(ctx, tc, xs, cwT, vwT, owT, outwT, bconv, bout,
                         ident, rdup, sid, y, dbg)
    nc.compile()
    return nc


def _kernel_body(ctx, tc, xs, cwT, vwT, owT, outwT, bconv, bout,
                 ident, rdup, sid, y, dbg):
    nc = tc.nc

    # ---- static SBUF tensors ----
    feat = nc.alloc_sbuf_tensor("feat", [C, HH * W], BF16)       # (c,(h,w))
    V = nc.alloc_sbuf_tensor("V", [W, HH, C], BF16)              # (w,(h,c))
    dcn = nc.alloc_sbuf_tensor("dcn", [C, HS * W], BF16)         # (c,(h,w))
    tb = nc.alloc_sbuf_tensor("tb", [W, 4, 3 * 72], BF16)        # tents
    ab = nc.alloc_sbuf_tensor("ab", [W, 4, 72], BF16)            # |o|
    may = nc.alloc_sbuf_tensor("may", [W, 4, OMP], BF16)         # (sy,g,k)
    # padded product buffer (g,dy5,dx5,slot9); 4 rotating, stay-zero slots
    U = [nc.alloc_sbuf_tensor(f"U{i}", [W, 900], BF16) for i in range(4)]
    # transposed bins with 2 zero pad cols each side (x-edge mask)
    binsT = nc.alloc_sbuf_tensor("binsT", [100, 4, 132], BF16)
    # shifted bins: [w, q, dx*20 + dy*4 + g]
    binsS = nc.alloc_sbuf_tensor("binsS", [W, 4, 100], BF16)
    prod = nc.alloc_sbuf_tensor("prod", [W, 2, 25 * C], BF16)    # tap slices
    omps = nc.alloc_psum_tensor("omps", [W, 4, OMP], F32)        # 4 rotating
    vopS = nc.alloc_psum_tensor("vopS", [W, 2, C], F32)
    btpS = nc.alloc_psum_tensor("btpS", [100, 2, C], F32)
    bspS = nc.alloc_psum_tensor("bspS", [W, 2, 100], F32)
    dpS = nc.alloc_psum_tensor("dpS", [C, 2, W], F32)
    wf32 = nc.alloc_sbuf_tensor("wf32", [C, C], F32)             # conv_w.T
    wbf = nc.alloc_sbuf_tensor("wbf", [C, 3 * C], BF16)          # vw|ow|outw|id
    wid = nc.alloc_sbuf_tensor("wid", [C, C], BF16)              # ident
    wrd = nc.alloc_sbuf_tensor("wrd", [100, 100], BF16)          # R shift
    wsid = nc.alloc_sbuf_tensor("wsid", [C, 5 * C], BF16)        # shifted ids
    bsb = nc.alloc_sbuf_tensor("bsb", [C, 3], F32)               # biases, ones

    cw_s = wf32.ap()[:, :]
    vw_s = wbf.ap()[:, 0:C]
    ow_s = wbf.ap()[:, C:C + OMP]
    outw_s = wbf.ap()[:, 2 * C:3 * C]
    id_s = wid.ap()[:, :]
    rd_s = wrd.ap()[:, :]
    sid_s = wsid.ap()
    nc.sync.dma_start(cw_s, cwT)
    nc.sync.dma_start(vw_s, vwT)
    nc.sync.dma_start(ow_s, owT)
    nc.sync.dma_start(outw_s, outwT)
    nc.sync.dma_start(id_s, ident)
    nc.sync.dma_start(rd_s, rdup)
    nc.sync.dma_start(sid_s[:, :], sid)
    nc.sync.dma_start(bsb.ap()[:, 0:1], bconv)
    nc.sync.dma_start(bsb.ap()[:, 1:2], bout)
    nc.vector.memset(bsb.ap()[:, 2:3], 1.0)

    # zero-init stay-zero buffers (once; unwritten slots stay zero)
    for u in U:
        nc.vector.memset(u.ap()[:, :], 0.0)
    nc.gpsimd.memset(binsT.ap()[:, :, :], 0.0)

    xpool = ctx.enter_context(tc.tile_pool(name="xin", bufs=3))
    cps_pool = ctx.enter_context(tc.tile_pool(name="cps", bufs=2, space="PSUM"))
    ypool = ctx.enter_context(tc.tile_pool(name="yout", bufs=3))

    # ---- stage A: conv GEMM (f32r, 1 cycle/row), 17 chunks of 4 rows ----
    CH = 512
    for i in range(HH * W // CH):
        xt = xpool.tile([C, CH], F32, tag="x")
        nc.sync.dma_start(xt[:, :], xs[:, 4 * i:4 * i + 4, :])
        cp = cps_pool.tile([C, CH], F32, tag="cps")
        nc.tensor.matmul(cp[:, :], cw_s, xt[:, :], start=True, stop=True)
        nc.scalar.activation(feat.ap()[:, i * CH:(i + 1) * CH], cp[:, :],
                             AF.Identity, bias=bsb.ap()[:, 0:1], scale=1.0)
    if dbg:
        nc.sync.dma_start(dbg["feat"], feat.ap()[:, :])

    # ---- per-row pipeline (software-pipelined stages) ----
    # stage offsets at iteration r (emission order minimizes within-iter
    # cross-engine gating: s0, s3, s4 first; s1/s2 produce for later iters):
    #   s0: V mm + evac row r;  om mm row r (om depends only on feat)
    #   s3: bins transpose/shift/dup row r-3 (inputs from iter r-1)
    #   s4: V-products + tap-sum + dcn evac row r-4 (binsS from iter r-1)
    #   s1: tents+may row r-1
    #   s2: P-products + bin reduce row r-2
    for r in range(HH):
        # --- s0: value GEMM for V row r; offset/mask GEMM for own row r ---
        fr = feat.ap()[:, r * W:(r + 1) * W]          # lhsT [ci, px=w]
        vop = vopS.ap()[:, r % 2, :]
        nc.tensor.matmul(vop, fr, vw_s, start=True, stop=True)
        nc.scalar.activation(V.ap()[:, r, :], vop, AF.Copy)
        if r < HS:
            fro = feat.ap()[:, (r + 2) * W:(r + 3) * W]
            nc.tensor.matmul(omps.ap()[:, r % 4, :], fro, ow_s,
                             start=True, stop=True)

        # --- s3: bin-sum transposes + shift/dup for row h3 ---
        h3 = r - 3
        if 0 <= h3 < HS:
            q = h3 % 4
            # btp[g*25+tap, w] = sum_slot U[w, g*225+tap*9+slot]: the PSUM
            # accumulation over 9 slot-transposes does the bin reduction
            btp = btpS.ap()[:, h3 % 2, :]
            u = U[q]
            for sl in range(9):
                lhsT = _ap(u, sl, [[900, W], [225, G], [9, 25]])
                nc.tensor.matmul(btp, lhsT, id_s,
                                 start=(sl == 0), stop=(sl == 8))
            nc.scalar.activation(binsT.ap()[0:100, q, 2:130], btp, AF.Copy)
            # binsS[w, dx*20+dy*4+g] = bins[w-dx+2, (g,dy,dx)]
            for dx in range(5):
                sh = dx - 2
                lhsT = _ap(binsT, q * 132 + (2 - sh), [[4 * 132, 100], [1, W]])
                nc.tensor.matmul(bspS.ap()[:, h3 % 2, dx * 20:(dx + 1) * 20],
                                 lhsT, rd_s[0:100, dx * 20:(dx + 1) * 20],
                                 start=True, stop=True)
            nc.scalar.activation(binsS.ap()[:, q, :], bspS.ap()[:, h3 % 2, :],
                                 AF.Copy)
            if dbg:
                nc.sync.dma_start(dbg["binss"][:, h3, :], binsS.ap()[:, q, :])

        # --- s4: V-products + PE tap-sum for own row h ---
        h = r - 4
        if 0 <= h < HS:
            q = h % 4
            hp = h % 2
            # prod[w, (dy*5+dx)*C + g*32 + c] = V[w, h+dy, g*32+c]
            #                                  * binsS[w, dx*20+dy*4+g]
            for dx in range(5):
                in0 = _ap(V, h * C,
                          [[HH * C, W], [C, 5], [32, G], [1, 32]])
                in1 = _ap(binsS, q * 100 + dx * 20,
                          [[4 * 100, W], [4, 5], [1, G], [0, 32]])
                outp = _ap(prod, hp * 25 * C + dx * C,
                           [[2 * 25 * C, W], [5 * C, 5], [32, G], [1, 32]])
                # dx=2 on Pool (frees DVE)
                eng = nc.gpsimd if dx == 2 else nc.vector
                eng.tensor_tensor(outp, in0, in1, op=ALU.mult)
            # PE sums the 25 tap slices via accumulating matmuls whose rhs
            # is a shifted identity: column w0 = w - (dx-2) un-shifts the
            # product back to its output pixel. dx=2 (full columns) first so
            # start=True initializes every PSUM column.
            dp = dpS.ap()[:, hp, :]
            ts = sorted(range(25), key=lambda t: t % 5 != 2)
            for i, t in enumerate(ts):
                dx = t % 5
                psl = _ap(prod, hp * 25 * C + t * C, [[2 * 25 * C, W], [1, C]])
                nc.tensor.matmul(dp, psl, sid_s[:, dx * C:(dx + 1) * C],
                                 start=(i == 0), stop=(i == 24))
            nc.scalar.activation(dcn.ap()[:, h * W:(h + 1) * W], dp, AF.Copy)

        # --- s1: tents + may for row h1 (consumed next iteration) ---
        h1 = r - 1
        if 0 <= h1 < HS:
            q = h1 % 4
            om = omps.ap()[:, q, :]   # [w, 108] PSUM: ox36|oy36|m36
            ps = 4 * OMP              # psum flat partition step
            # t- = relu(-o) ; t+ = relu(o) ; t0 = 1-|o| (|o|<1 guaranteed)
            nc.scalar.activation(tb.ap()[:, q, 0:72], om[:, 0:72], AF.Relu,
                                 scale=-1.0)
            nc.scalar.activation(tb.ap()[:, q, 144:216], om[:, 0:72], AF.Relu,
                                 scale=1.0)
            nc.scalar.activation(ab.ap()[:, q, :], om[:, 0:72], AF.Abs)
            nc.scalar.activation(tb.ap()[:, q, 72:144], ab.ap()[:, q, :],
                                 AF.Identity, bias=bsb.ap()[:, 2:3], scale=-1.0)
            # may[sy,g,k] = m * t_y[sy]   (DVE; PSUM mask src)
            in0 = _ap(tb, q * 216 + 36, [[4 * 216, W], [72, 3], [9, G], [1, 9]])
            in1 = _ap(omps, q * OMP + 72, [[ps, W], [0, 3], [9, G], [1, 9]])
            outp = _ap(may, q * OMP, [[4 * OMP, W], [36, 3], [9, G], [1, 9]])
            nc.vector.tensor_tensor(outp, in0, in1, op=ALU.mult)

        # --- s2: stencil binning for row h2 (consumed next iteration) ---
        h2 = r - 2
        if 0 <= h2 < HS:
            q = h2 % 4
            # P[g,ky,kx,sx] = may[sy] * t_x[sx] -> U padded (g,dy5,dx5,slot9)
            # U slot: g*225 + (ky+sy)*45 + (kx+sx)*9 + ky*3 + kx
            u = U[q]
            for sy in range(3):
                for ky in range(3):
                    in0 = _ap(may, q * OMP + sy * 36 + ky * 3,
                              [[4 * OMP, W], [9, G], [1, 3], [0, 3]])
                    in1 = _ap(tb, q * 216 + ky * 3,
                              [[4 * 216, W], [9, G], [1, 3], [72, 3]])
                    outp = _ap(u, sy * 45 + ky * 48,
                               [[900, W], [225, G], [10, 3], [9, 3]])
                    nc.gpsimd.tensor_tensor(outp, in0, in1, op=ALU.mult)

    if dbg:
        nc.sync.dma_start(dbg["v"], V.ap()[:, :, :])
        nc.sync.dma_start(dbg["dcn"], dcn.ap()[:, :])

    # ---- out projection (bf16, 1 cycle/row) ----
    for i in range(HS * W // CH):
        yp = cps_pool.tile([C, CH], F32, tag="cps")
        nc.tensor.matmul(yp[:, :], outw_s, dcn.ap()[:, i * CH:(i + 1) * CH],
                         start=True, stop=True)
        yt = ypool.tile([C, CH], F32, tag="y")
        nc.scalar.activation(yt[:, :], yp[:, :], AF.Identity,
                             bias=bsb.ap()[:, 1:2], scale=1.0)
        nc.sync.dma_start(y[:, 4 * i:4 * i + 4, :], yt[:, :])


def _prep_inputs(x, conv_w, conv_b, value_w, value_b, om_w, om_b, out_w, out_b):
    from ml_dtypes import bfloat16
    omperm = ([g * 27 + 2 * k for g in range(G) for k in range(K)]
              + [g * 27 + 2 * k + 1 for g in range(G) for k in range(K)]
              + [g * 27 + 18 + k for g in range(G) for k in range(K)])
    assert np.all(om_b[omperm] == 0.0), "nonzero om bias not supported"
    assert np.all(value_b == 0.0), "nonzero value bias not supported"
    owT = np.ascontiguousarray(om_w[omperm].T.astype(bfloat16))
    # R shift/relayout matrix: R[g*25 + 5*dy + dx, dx*20 + dy*4 + g] = 1
    rd = np.zeros((100, 100), np.float32)
    for dx in range(5):
        for dy in range(5):
            for g in range(G):
                rd[g * 25 + 5 * dy + dx, dx * 20 + dy * 4 + g] = 1.0
    # shifted identities: Sid[w, dx*C + w0] = 1 iff w0 = w - (dx-2)
    sid = np.zeros((C, 5 * C), np.float32)
    for dx in range(5):
        sh = dx - 2
        for w in range(C):
            w0 = w - sh
            if 0 <= w0 < C:
                sid[w, dx * C + w0] = 1.0
    common = dict(
        cwT=np.ascontiguousarray(conv_w.T.astype(np.float32)),
        vwT=np.ascontiguousarray(value_w.T.astype(bfloat16)),
        owT=owT,
        outwT=np.ascontiguousarray(out_w.T.astype(bfloat16)),
        bconv=np.ascontiguousarray(conv_b.astype(np.float32).reshape(C, 1)),
        bout=np.ascontiguousarray(out_b.astype(np.float32).reshape(C, 1)),
        ident=np.eye(C, dtype=bfloat16),
        rdup=rd.astype(bfloat16),
        sid=sid.astype(bfloat16),
    )
    in_maps = []
    for core in range(NCORES):
        n, half = core // 2, core % 2
        h0 = half * HS
        xsl = np.zeros((C, HH, W), np.float32)
        lo, hi = h0 - 2, h0 + HS + 2
        clo, chi = max(0, lo), min(H, hi)
        xsl[:, clo - lo:chi - lo, :] = x[n, :, clo:chi, :]
        m = dict(common)
        m["xs"] = xsl
        in_maps.append(m)
    return in_maps


def kernel(**inputs):
    inputs = {k: np.asarray(v) for k, v in inputs.items()}
    x = inputs["x"]
    if "prog" not in _CACHE:
        _CACHE["prog"] = _build_program(debug=False)
    nc = _CACHE["prog"]
    in_maps = _prep_inputs(
        x, inputs["conv_w"], inputs["conv_b"], inputs["value_w"],
        inputs["value_b"], inputs["om_w"], inputs["om_b"], inputs["out_w"],
        inputs["out_b"])
    res = run_bass_kernel_spmd(nc, in_maps, core_ids=list(range(NCORES)))
    out = np.empty((N, C, H, W), np.float32)
    for core in range(NCORES):
        n, half = core // 2, core % 2
        out[n, :, half * HS:(half + 1) * HS, :] = res.results[core]["y"]
    return out

